# revision 18
# baseline (speedup 1.0000x reference)
"""Trainium2 Bass kernel for dense transformer block nn_Block_68221260529679.

Layout: B=2, T=2048, D=2048, N=8 q-heads, K=1 kv-head, H=256, F=16384.

Sharding (8 NeuronCores): DP over batch (2 groups of 4) x sequence parallel
within each group.  T is split into 16 row-tiles of 128; rank r in a group
owns tiles G[r] = [r, 4+r, 11-r, 15-r] (interleaved so causal-attention work
is identical across ranks: sum(g+1) == 34 for every rank).

Each core, for its 512 rows:
  - rmsnorm(x) -> hT, project k/v for OWN rows, rope -> ONE AllGather of
    packed [k|v] across the 4-core group (the only collective),
  - project q (all 8 heads) + rope while the AllGather flies,
  - causal attention for all 8 heads over gathered K/V.  The per-slot key
    length is fixed at [512,1024,1536,2048] so the instruction stream is
    rank-independent; rank-specific causal boundaries come in as additive
    bf16 mask data from the host,
  - o-proj (full head contraction, no collective) + residual -> x2,
  - rmsnorm -> h2T, then the FULL MLP (all of F) for its own rows, streaming
    w_gating/w_linear from HBM in 8 F-chunks; down-proj partials accumulate
    in fp32 into x2 -> out rows.

All matmuls bf16 with fp32 PSUM accumulation; norms/softmax/residual fp32.
(1+scale) rmsnorm factors and the q H^-0.5 are folded into weights host-side;
rope sin/cos and causal masks are precomputed host-side per core.
"""

from contextlib import ExitStack

import numpy as np
import ml_dtypes

import concourse.bass as bass
import concourse.mybir as mybir
import concourse.tile as tile
from concourse import bacc
from concourse.masks import make_identity

F32 = mybir.dt.float32
BF16 = mybir.dt.bfloat16
AF = mybir.ActivationFunctionType
ALU = mybir.AluOpType
BIG_NEG = -2.3819763e38
GROUPS = [[0, 1, 2, 3], [4, 5, 6, 7]]

FULL_CFG = dict(T=2048, D=2048, H=256, NH=8, F=16384)

# rank r owns global row-tiles TILE_MAP[r]; slot j's key length is SMAX[j]
TILE_MAP = [[r, 4 + r, 11 - r, 15 - r] for r in range(4)]
SMAX = [512, 1024, 1536, 2048]
# additive mask only for the diagonal 512-col chunk of each slot
MASK_COLS = 4 * 512


def build(cfg):
    T, D, H, NH, F = cfg["T"], cfg["D"], cfg["H"], cfg["NH"], cfg["F"]
    REPS = cfg.get("reps", 1)
    assert H == 256 and T == 2048 and D == 2048
    DT = D // 128            # 16 contraction blocks
    NT = 4                   # own row-tiles per core
    ROWS = NT * 128          # 512 own rows
    QB = NH * H // 128       # 16 q/o col blocks
    GT = T // 128            # 16 global row tiles
    FQ = 8                   # F streamed in 8 chunks
    FBH = F // 128 // FQ     # 16 f-blocks per chunk
    DCH = 4                  # D split for down-proj / o-proj
    DC = D // DCH            # 512

    nc = bacc.Bacc("TRN2", target_bir_lowering=False, debug=False,
                   num_devices=8)
    x_ext = nc.dram_tensor("xin", [ROWS, D], F32, kind="ExternalInput").ap()
    wq_ext = nc.dram_tensor("wq", [NH, 128, DT, 2 * 128], BF16,
                            kind="ExternalInput").ap()
    wkv_ext = nc.dram_tensor("wkv", [128, DT, 2 * H], BF16,
                             kind="ExternalInput").ap()
    wo_ext = nc.dram_tensor("wo", [DCH, 128, QB, DC], BF16,
                            kind="ExternalInput").ap()
    wg_ext = nc.dram_tensor("wg", [F // 128, 128, DT, 256], BF16,
                            kind="ExternalInput").ap()
    wl_ext = nc.dram_tensor("wl", [FQ, DCH, 128, FBH, DC], BF16,
                            kind="ExternalInput").ap()
    sinq_ext = nc.dram_tensor("sinq", [128, ROWS], F32,
                              kind="ExternalInput").ap()
    cosq_ext = nc.dram_tensor("cosq", [128, ROWS], F32,
                              kind="ExternalInput").ap()
    sink_ext = nc.dram_tensor("sink", [128, NT, 128], F32,
                              kind="ExternalInput").ap()
    cosk_ext = nc.dram_tensor("cosk", [128, NT, 128], F32,
                              kind="ExternalInput").ap()
    mask_ext = nc.dram_tensor("mask", [128, MASK_COLS], BF16,
                              kind="ExternalInput").ap()
    out_ext = nc.dram_tensor("out", [ROWS, D], F32, kind="ExternalOutput").ap()

    with tile.TileContext(nc) as tc, ExitStack() as top:
        cons = top.enter_context(tc.tile_pool(name="cons", bufs=1))
        dram = top.enter_context(tc.tile_pool(name="dram", bufs=1,
                                              space="DRAM"))

        ident = cons.tile([128, 128], BF16)
        make_identity(nc, ident)
        eps = cons.tile([128, 1], F32)
        nc.vector.memset(eps, 1e-6)
        # tables are declared here but DMA'd after the first x/wkv loads so
        # they don't delay the critical path at kernel start
        sinq = cons.tile([128, ROWS], F32)
        cosq = cons.tile([128, ROWS], F32)
        sink = cons.tile([128, NT, 128], F32)
        cosk = cons.tile([128, NT, 128], F32)
        mask = cons.tile([128, MASK_COLS], BF16)

        kv_own = dram.tile([ROWS, 2 * H], BF16, tag="kv_own", name="kv_own")
        kv_all = dram.tile([T, 2 * H], BF16, tag="kv_all", name="kv_all")

        for _rep in range(REPS):
            with ExitStack() as rep_sc:
                per = rep_sc.enter_context(tc.tile_pool(name="per", bufs=1))
                x2_sb = per.tile([128, NT, D], F32, tag="x2")
                h2T = per.tile([128, DT, ROWS], BF16, tag="h2T")
                qT = per.tile([128, QB, ROWS], BF16, tag="qT")

                # ======== phase A1: norms + q/k/v projections ========
                with (
                    tc.tile_pool(name="pa1", bufs=2) as pa,
                    tc.tile_pool(name="ps1", bufs=2, space="PSUM") as ps,
                ):
                    # ---------- P0: rmsnorm(x) -> hT ----------
                    hT = pa.tile([128, DT, ROWS], BF16, tag="hT", bufs=1)
                    for j in range(NT):
                        jsl = slice(j * 128, (j + 1) * 128)
                        xt = pa.tile([128, D], F32, tag="xt", bufs=1)
                        nc.sync.dma_start(out=xt, in_=x_ext[jsl])
                        h = pa.tile([128, D], BF16, tag="h")
                        ssq = pa.tile([128, 1], F32, tag="ssq")
                        nc.scalar.activation(out=h, in_=xt, func=AF.Square,
                                             accum_out=ssq)
                        rstd = pa.tile([128, 1], F32, tag="rstd")
                        nc.scalar.activation(out=rstd, in_=ssq, func=AF.Sqrt,
                                             bias=eps, scale=1.0 / D)
                        nc.vector.reciprocal(out=rstd, in_=rstd)
                        nc.vector.tensor_scalar_mul(h, xt, rstd)
                        for kd in range(DT):
                            pt = ps.tile([128, 128], BF16, tag="tp")
                            nc.tensor.transpose(
                                pt, h[:, kd * 128:(kd + 1) * 128], ident)
                            nc.vector.tensor_copy(hT[:, kd, jsl], pt)

                    # ---------- P1: k/v for own rows + rope, AllGather ----
                    wkvs = pa.tile([128, DT, 2 * H], BF16, tag="wkvs",
                                   bufs=1)
                    nc.sync.dma_start(out=wkvs, in_=wkv_ext)
                    nc.sync.dma_start(out=sink, in_=sink_ext)
                    nc.sync.dma_start(out=cosk, in_=cosk_ext)
                    kv_st = pa.tile([128, NT, 2 * H], BF16, tag="kvst",
                                    bufs=1)
                    for j in range(NT):
                        jsl = slice(j * 128, (j + 1) * 128)
                        # fused k|v projection: one N=512 chain per tile
                        pk = ps.tile([128, 512], F32, tag="mm512", bufs=3)
                        for kd in range(DT):
                            nc.tensor.matmul(pk, hT[:, kd, jsl],
                                             wkvs[:, kd, :],
                                             start=kd == 0, stop=kd == DT - 1)
                        # rope natural layout: x1 = cols 0:128, x2 = 128:256
                        cs, sn = cosk[:, j], sink[:, j]
                        t1 = pa.tile([128, 128], F32, tag="rp1")
                        t2 = pa.tile([128, 128], F32, tag="rp2")
                        nc.vector.tensor_tensor(t1, pk[:, 0:128], cs,
                                                op=ALU.mult)
                        nc.vector.tensor_tensor(t2, pk[:, 128:256], sn,
                                                op=ALU.mult)
                        nc.vector.tensor_tensor(kv_st[:, j, 0:128], t1, t2,
                                                op=ALU.subtract)
                        nc.vector.tensor_tensor(t1, pk[:, 128:256], cs,
                                                op=ALU.mult)
                        nc.vector.tensor_tensor(t2, pk[:, 0:128], sn,
                                                op=ALU.mult)
                        nc.vector.tensor_tensor(kv_st[:, j, 128:256], t1, t2,
                                                op=ALU.add)
                        nc.vector.tensor_copy(kv_st[:, j, 256:512],
                                              pk[:, 256:512])
                        nc.sync.dma_start(out=kv_own[jsl], in_=kv_st[:, j])
                    nc.gpsimd.collective_compute(
                        "AllGather", ALU.bypass, replica_groups=GROUPS,
                        ins=[kv_own.opt()], outs=[kv_all.opt()])

                    # ---------- P2: q proj + rope (overlaps AllGather) ----
                    nc.sync.dma_start(out=sinq, in_=sinq_ext)
                    nc.sync.dma_start(out=cosq, in_=cosq_ext)
                    nc.sync.dma_start(out=mask, in_=mask_ext)
                    for hd in range(NH):
                        wqh = pa.tile([128, DT, 2 * 128], BF16, tag="wqh")
                        nc.sync.dma_start(out=wqh, in_=wq_ext[hd])
                        p1 = ps.tile([128, 512], F32, tag="mm512", bufs=3)
                        p2 = ps.tile([128, 512], F32, tag="mm512", bufs=3)
                        for kd in range(DT):
                            nc.tensor.matmul(p1, wqh[:, kd, 0:128],
                                             hT[:, kd, :],
                                             start=kd == 0, stop=kd == DT - 1)
                        for kd in range(DT):
                            nc.tensor.matmul(p2, wqh[:, kd, 128:256],
                                             hT[:, kd, :],
                                             start=kd == 0, stop=kd == DT - 1)
                        t1 = pa.tile([128, ROWS], F32, tag="rq1")
                        t2 = pa.tile([128, ROWS], F32, tag="rq2")
                        nc.vector.tensor_tensor(t1, p1, cosq, op=ALU.mult)
                        nc.vector.tensor_tensor(t2, p2, sinq, op=ALU.mult)
                        nc.vector.tensor_tensor(qT[:, 2 * hd], t1, t2,
                                                op=ALU.subtract)
                        nc.vector.tensor_tensor(t1, p2, cosq, op=ALU.mult)
                        nc.vector.tensor_tensor(t2, p1, sinq, op=ALU.mult)
                        nc.vector.tensor_tensor(qT[:, 2 * hd + 1], t1, t2,
                                                op=ALU.add)

                # ======== phase A2: attention + o-proj + norm2 ========
                with (
                    tc.tile_pool(name="pa2", bufs=2) as pa,
                    tc.tile_pool(name="ps2", bufs=2, space="PSUM") as ps,
                ):
                    # ---------- P3: pull gathered K/V into SBUF ----------
                    rowof = {}
                    for q in range(4):
                        for jj, g in enumerate(TILE_MAP[q]):
                            rowof[g] = q * ROWS + jj * 128
                    v_sb = pa.tile([128, GT, H], BF16, tag="v", bufs=1)
                    kT = pa.tile([128, 2, T], BF16, tag="kT", bufs=1)
                    for g in range(GT):
                        r0 = rowof[g]
                        ks = pa.tile([128, H], BF16, tag="ks")
                        nc.sync.dma_start(out=ks,
                                          in_=kv_all[r0:r0 + 128, 0:H])
                        for m in range(2):
                            pt = ps.tile([128, 128], BF16, tag="tp")
                            nc.tensor.transpose(
                                pt, ks[:, m * 128:(m + 1) * 128], ident)
                            nc.vector.tensor_copy(
                                kT[:, m, g * 128:(g + 1) * 128], pt)
                    for g in range(GT):
                        r0 = rowof[g]
                        nc.sync.dma_start(out=v_sb[:, g],
                                          in_=kv_all[r0:r0 + 128, H:2 * H])

                    # ---------- P4: attention ----------
                    # no max-subtraction: |logits| <= |q||k| stays far below
                    # the fp32 exp overflow threshold for this block's scales
                    encT = pa.tile([128, QB, ROWS], BF16, tag="encT",
                                   bufs=1)
                    for j in range(NT):
                        jsl = slice(j * 128, (j + 1) * 128)
                        S = SMAX[j]
                        nS = S // 128
                        nsc = S // 512
                        for hd in range(NH):
                            pb = pa.tile([128, SMAX[-1]], BF16, tag="pb")
                            sums = []
                            for sc in range(nsc):
                                ssl = slice(sc * 512, (sc + 1) * 512)
                                pl = ps.tile([128, 512], F32, tag="mm512",
                                             bufs=3)
                                nc.tensor.matmul(pl, qT[:, 2 * hd, jsl],
                                                 kT[:, 0, ssl],
                                                 start=True, stop=False)
                                nc.tensor.matmul(pl, qT[:, 2 * hd + 1, jsl],
                                                 kT[:, 1, ssl],
                                                 start=False, stop=True)
                                sume = pa.tile([128, 1], F32,
                                               tag=f"sume{sc}")
                                if sc == j:
                                    # diagonal chunk: add causal mask first
                                    lgm = pa.tile([128, 512], F32, tag="lgm")
                                    nc.vector.tensor_tensor(
                                        lgm, pl, mask[:, j * 512:
                                                      (j + 1) * 512],
                                        op=ALU.add)
                                    nc.scalar.activation(
                                        out=pb[:, ssl], in_=lgm,
                                        func=AF.Exp, accum_out=sume)
                                else:
                                    # fully-causal chunk: exp straight
                                    # from PSUM
                                    nc.scalar.activation(
                                        out=pb[:, ssl], in_=pl,
                                        func=AF.Exp, accum_out=sume)
                                sums.append(sume)
                            tot = sums[0]
                            for s2 in sums[1:]:
                                nc.vector.tensor_tensor(tot, tot, s2,
                                                        op=ALU.add)
                            rsum = pa.tile([128, 1], F32, tag="rsum")
                            nc.vector.reciprocal(rsum, tot)
                            nc.vector.tensor_scalar_mul(pb[:, :S], pb[:, :S],
                                                        rsum)
                            pT = pa.tile([128, GT, 128], BF16, tag="pT")
                            for s in range(nS):
                                ptp = ps.tile([128, 128], BF16, tag="tp")
                                nc.tensor.transpose(
                                    ptp, pb[:, s * 128:(s + 1) * 128], ident)
                                nc.vector.tensor_copy(pT[:, s], ptp)
                            for m in range(2):
                                pe = ps.tile([128, 128], F32, tag="av")
                                for s in range(nS):
                                    nc.tensor.matmul(
                                        pe,
                                        v_sb[:, s, m * 128:(m + 1) * 128],
                                        pT[:, s],
                                        start=s == 0, stop=s == nS - 1)
                                nc.vector.tensor_copy(
                                    encT[:, 2 * hd + m, jsl], pe)

                    # ---------- P4b: o-proj + residual ----------
                    for dch in range(DCH):
                        dsl = slice(dch * DC, (dch + 1) * DC)
                        wod = pa.tile([128, QB, DC], BF16, tag="wod")
                        nc.sync.dma_start(out=wod, in_=wo_ext[dch])
                        for j in range(NT):
                            jsl = slice(j * 128, (j + 1) * 128)
                            po = ps.tile([128, 512], F32, tag="mm512",
                                         bufs=3)
                            for blk in range(QB):
                                nc.tensor.matmul(po, encT[:, blk, jsl],
                                                 wod[:, blk],
                                                 start=blk == 0,
                                                 stop=blk == QB - 1)
                            xs = pa.tile([128, DC], F32, tag="xs")
                            nc.sync.dma_start(out=xs, in_=x_ext[jsl, dsl])
                            nc.vector.tensor_tensor(x2_sb[:, j, dsl], po, xs,
                                                    op=ALU.add)

                    # ---------- P5: rmsnorm(x2) -> h2T ----------
                    for j in range(NT):
                        jsl = slice(j * 128, (j + 1) * 128)
                        h2 = pa.tile([128, D], BF16, tag="h2n")
                        ssq = pa.tile([128, 1], F32, tag="ssq")
                        nc.scalar.activation(out=h2, in_=x2_sb[:, j],
                                             func=AF.Square, accum_out=ssq)
                        rstd = pa.tile([128, 1], F32, tag="rstd")
                        nc.scalar.activation(out=rstd, in_=ssq, func=AF.Sqrt,
                                             bias=eps, scale=1.0 / D)
                        nc.vector.reciprocal(out=rstd, in_=rstd)
                        nc.vector.tensor_scalar_mul(h2, x2_sb[:, j], rstd)
                        for kd in range(DT):
                            pt = ps.tile([128, 128], BF16, tag="tp")
                            nc.tensor.transpose(
                                pt, h2[:, kd * 128:(kd + 1) * 128], ident)
                            nc.vector.tensor_copy(h2T[:, kd, jsl], pt)

                # ---------- P6: MLP, F streamed in FQ chunks ----------
                with (
                    tc.tile_pool(name="pb", bufs=2) as pb_,
                    tc.tile_pool(name="psb", bufs=2, space="PSUM") as psb,
                ):
                    for qf in range(FQ):
                        ffT = pb_.tile([128, FBH, ROWS], BF16, tag="ffT",
                                       bufs=2)
                        for fb in range(FBH):
                            fg = qf * FBH + fb
                            wgf = pb_.tile([128, DT, 256], BF16, tag="wgf",
                                           bufs=4)
                            nc.sync.dma_start(out=wgf, in_=wg_ext[fg])
                            gps = psb.tile([128, ROWS], F32, tag="gps")
                            ups = psb.tile([128, ROWS], F32, tag="ups")
                            for kd in range(DT):
                                nc.tensor.matmul(gps, wgf[:, kd, 0:128],
                                                 h2T[:, kd, :],
                                                 start=kd == 0,
                                                 stop=kd == DT - 1)
                            for kd in range(DT):
                                nc.tensor.matmul(ups, wgf[:, kd, 128:256],
                                                 h2T[:, kd, :],
                                                 start=kd == 0,
                                                 stop=kd == DT - 1)
                            ga = pb_.tile([128, ROWS], F32, tag="ga")
                            nc.scalar.activation(out=ga, in_=gps,
                                                 func=AF.Gelu_apprx_tanh)
                            nc.vector.tensor_tensor(ffT[:, fb], ga, ups,
                                                    op=ALU.mult)
                        for dch in range(DCH):
                            dsl = slice(dch * DC, (dch + 1) * DC)
                            wlc = pb_.tile([128, FBH, DC], BF16, tag="wlc",
                                           bufs=3)
                            nc.sync.dma_start(out=wlc, in_=wl_ext[qf, dch])
                            for j in range(NT):
                                jsl = slice(j * 128, (j + 1) * 128)
                                dps = psb.tile([128, DC], F32, tag="dps")
                                for fb in range(FBH):
                                    nc.tensor.matmul(
                                        dps, ffT[:, fb, jsl], wlc[:, fb],
                                        start=fb == 0, stop=fb == FBH - 1)
                                nc.vector.tensor_tensor(
                                    x2_sb[:, j, dsl], x2_sb[:, j, dsl], dps,
                                    op=ALU.add)
                    # ---------- P7: write out ----------
                    for j in range(NT):
                        nc.sync.dma_start(
                            out=out_ext[j * 128:(j + 1) * 128],
                            in_=x2_sb[:, j])
    nc.compile()
    return nc


# ---------------------------------------------------------------------------
# host side
# ---------------------------------------------------------------------------

def _pa(w, inner=128):
    """[A*128, N] -> [128, A, N] partition-major layout."""
    a = w.shape[0] // inner
    return np.ascontiguousarray(
        w.reshape(a, inner, w.shape[1]).transpose(1, 0, 2))


def make_in_maps(cfg, x, positions, attn_mask, scale_attn, w_q, w_kv, w_o,
                 scale_ffn, w_gating, w_linear):
    T, D, H, NH, F = cfg["T"], cfg["D"], cfg["H"], cfg["NH"], cfg["F"]
    DT, QB = D // 128, NH * H // 128
    FQ, FBH, DCH, DC = 8, F // 128 // 8, 4, D // 4
    bf = ml_dtypes.bfloat16
    s1a = (1.0 + np.asarray(scale_attn, np.float32))[:, None]
    s1f = (1.0 + np.asarray(scale_ffn, np.float32))[:, None]

    # shared (rank-independent) weights
    # wq: [NH, 128, DT, 256]
    wq_h = np.stack([
        _pa((np.asarray(w_q[n], np.float32) * s1a * H ** -0.5).astype(bf))
        for n in range(NH)])
    k_w = np.asarray(w_kv[0, 0], np.float32) * s1a
    v_w = np.asarray(w_kv[1, 0], np.float32) * s1a
    wkv_h = _pa(np.concatenate([k_w, v_w], axis=1).astype(bf))
    # wo: [DCH, 128, QB, DC]
    wo_cat = np.concatenate(list(np.asarray(w_o, np.float32)),
                            axis=0).astype(bf)       # [NH*H, D]
    wo_h = np.ascontiguousarray(
        wo_cat.reshape(QB, 128, DCH, DC).transpose(2, 1, 0, 3))
    gate = (np.asarray(w_gating[0], np.float32) * s1f).astype(bf)
    up = (np.asarray(w_gating[1], np.float32) * s1f).astype(bf)
    # wg: [F/128, 128, DT, 256]
    gate = gate.reshape(DT, 128, F // 128, 128).transpose(2, 1, 0, 3)
    up = up.reshape(DT, 128, F // 128, 128).transpose(2, 1, 0, 3)
    wg_h = np.ascontiguousarray(np.concatenate([gate, up], axis=3))
    # wl: [FQ, DCH, 128, FBH, DC]
    wl_h = np.ascontiguousarray(
        np.asarray(w_linear, np.float32).astype(bf)
        .reshape(FQ, FBH, 128, DCH, DC).transpose(0, 3, 2, 1, 4))

    freq = 10000.0 ** (2.0 / H * np.arange(H // 2, dtype=np.float32))
    in_maps = []
    for c in range(8):
        b, r = divmod(c, 4)
        tiles = TILE_MAP[r]
        gidx = np.concatenate([np.arange(g * 128, (g + 1) * 128)
                               for g in tiles])              # own global rows
        xb = np.asarray(x[b], np.float32)
        pos = np.asarray(positions[b], np.float32)[gidx]     # [512]
        rad_q = pos[None, :] / freq[:, None]                 # [128, 512]
        rad_k = (pos.reshape(4, 128)[None, :, :]
                 / freq[:, None, None])                      # [i, j, p]
        rad_k = np.ascontiguousarray(rad_k.transpose(2, 1, 0))  # [p, j, i]
        # additive causal mask for each slot's diagonal 512-col chunk
        mk = np.full((128, MASK_COLS), BIG_NEG, np.float32)
        for j, g in enumerate(tiles):
            rows_g = g * 128 + np.arange(128)
            cols = j * 512 + np.arange(512)
            mk[:, j * 512:(j + 1) * 512] = np.where(
                cols[None, :] <= rows_g[:, None], 0.0, BIG_NEG)
        in_maps.append({
            "xin": np.ascontiguousarray(xb[gidx]),
            "wq": wq_h, "wkv": wkv_h, "wo": wo_h, "wg": wg_h, "wl": wl_h,
            "sinq": np.ascontiguousarray(np.sin(rad_q)),
            "cosq": np.ascontiguousarray(np.cos(rad_q)),
            "sink": np.ascontiguousarray(np.sin(rad_k)),
            "cosk": np.ascontiguousarray(np.cos(rad_k)),
            "mask": mk.astype(bf),
        })
    return in_maps


def assemble(cfg, results, B):
    T, D = cfg["T"], cfg["D"]
    out = np.empty((B, T, D), np.float32)
    for c in range(8):
        b, r = divmod(c, 4)
        res = results[c]["out"]
        for j, g in enumerate(TILE_MAP[r]):
            out[b, g * 128:(g + 1) * 128] = res[j * 128:(j + 1) * 128]
    return out


# cached compiled program + jitted runner -----------------------------------

_CACHE = {}


def _get_runner(cfg_key, cfg):
    if cfg_key in _CACHE:
        return _CACHE[cfg_key]
    runner = _runner_from_nc(build(cfg))
    _CACHE[cfg_key] = runner
    return runner


def _runner_from_nc(nc):
    import jax
    from jax.experimental.shard_map import shard_map
    from jax.sharding import Mesh, NamedSharding, PartitionSpec
    from concourse import bass2jax

    bass2jax.install_neuronx_cc_hook()

    partition_name = (nc.partition_id_tensor.name
                      if nc.partition_id_tensor else None)
    in_names, out_names, out_avals, zero_shapes = [], [], [], []
    for alloc in nc.m.functions[0].allocations:
        if not isinstance(alloc, mybir.MemoryLocationSet):
            continue
        name = alloc.memorylocations[0].name
        if alloc.kind == "ExternalInput":
            if name != partition_name:
                in_names.append(name)
        elif alloc.kind == "ExternalOutput":
            out_names.append(name)
            shape = tuple(alloc.tensor_shape)
            dtype = mybir.dt.np(alloc.dtype)
            out_avals.append(jax.core.ShapedArray(shape, dtype))
            zero_shapes.append((shape, dtype))
    n_params = len(in_names)
    all_in_names = in_names + out_names
    if partition_name is not None:
        all_in_names = all_in_names + [partition_name]

    def _body(*args):
        operands = list(args)
        if partition_name is not None:
            operands.append(bass2jax.partition_id_tensor())
        outs = bass2jax._bass_exec_p.bind(
            *operands,
            out_avals=tuple(out_avals),
            in_names=tuple(all_in_names),
            out_names=tuple(out_names),
            lowering_input_output_aliases=(),
            sim_require_finite=True,
            sim_require_nnan=True,
            nc=nc,
        )
        return tuple(outs)

    n_outs = len(out_names)
    donate = tuple(range(n_params, n_params + n_outs))
    devices = jax.devices()[:8]
    mesh = Mesh(np.asarray(devices), ("core",))
    in_specs = (PartitionSpec("core"),) * (n_params + n_outs)
    out_specs = (PartitionSpec("core"),) * n_outs
    sh = NamedSharding(mesh, PartitionSpec("core"))
    in_allocs = {}
    for alloc in nc.m.functions[0].allocations:
        if isinstance(alloc, mybir.MemoryLocationSet):
            in_allocs[alloc.memorylocations[0].name] = alloc
    arg_structs = []
    for name in in_names:
        al = in_allocs[name]
        shape = tuple(al.tensor_shape)
        arg_structs.append(jax.ShapeDtypeStruct(
            (8 * shape[0],) + shape[1:], mybir.dt.np(al.dtype), sharding=sh))
    for shape, dtype in zero_shapes:
        arg_structs.append(jax.ShapeDtypeStruct(
            (8 * shape[0],) + shape[1:], dtype, sharding=sh))

    def compile_fn():
        jitted = jax.jit(
            shard_map(_body, mesh=mesh, in_specs=in_specs,
                      out_specs=out_specs, check_rep=False),
            donate_argnums=donate, keep_unused=True)
        return jitted.lower(*arg_structs).compile()

    sharded = bass2jax.fast_dispatch_compile(compile_fn)

    class Runner:
        pass

    runner = Runner()
    runner.sharded = sharded
    runner.mesh = mesh
    runner.in_names = in_names
    runner.out_names = out_names
    runner.out_avals = out_avals
    runner.zero_shapes = zero_shapes

    def concat_inputs(in_maps):
        return [np.concatenate([np.asarray(m[name]) for m in in_maps],
                               axis=0) for name in in_names]

    def make_zeros():
        return [np.zeros((8 * s[0], *s[1:]), d) for s, d in zero_shapes]

    def split_outputs(out_arrs):
        return [
            {name: np.asarray(out_arrs[i]).reshape(8, *out_avals[i].shape)[c]
             for i, name in enumerate(out_names)}
            for c in range(8)
        ]

    runner.concat_inputs = concat_inputs
    runner.make_zeros = make_zeros
    runner.split_outputs = split_outputs

    def run(in_maps):
        out_arrs = sharded(*concat_inputs(in_maps), *make_zeros())
        return split_outputs(out_arrs)

    runner.run = run
    return runner


def run_cfg(cfg, inputs):
    cfg_key = tuple(sorted(cfg.items()))
    runner = _get_runner(cfg_key, cfg)
    in_maps = make_in_maps(cfg, **inputs)
    results = runner.run(in_maps)
    return assemble(cfg, results, np.asarray(inputs["x"]).shape[0])


def kernel(**inputs):
    return run_cfg(FULL_CFG, inputs)
